# revision 6
# baseline (speedup 1.0000x reference)
"""Trainium2 Bass kernel v4: 16-head MHA (S=2048, D=1024, Dk=Dv=64) on 8 cores.

Sharding: tensor-parallel over heads (2 heads/core) + row-sharded Wo
(fp-summed partials on host). Differences vs v3 baseline:

  - Both heads processed per phase (phase = query half). Score matmuls are
    K=64 ROW-TILED PAIRS (tile_position (0,0)/(64,0)) so the two heads'
    QK^T share the PE array concurrently -> 2x score throughput.
  - AV matmuls are M=64 COL-TILED PAIRS ((0,0)/(0,64)); the softmax
    denominator moves out of the AV stationary (no more M=65) into a
    4-tile M=1 "denominator quad" slot per t-block ((0,0)/(0,32)/(0,64)/
    (0,96)) -> AV+denominator costs 3 slots per t-block instead of 4.
  - QKV projection chunks are interleaved INTO the attention steps so the
    exp stream (the scarce resource: ACT ~114 G/s + DVE ~95 G/s) starts
    ~10us earlier and the PE never idles long enough for HAM to
    re-throttle the clock.
  - exp split ACT (table exp) / DVE (Schraudolph int16 bitcast) per-step.
  - Denominator reciprocal via one reciprocal_approx_fast over the quad
    psum rows (no DMA round-trip), broadcast by K=1 fp32 matmuls.
  - Output projection partials evacuated bf16 (ACT/DVE alternating),
    DMA'd per chunk; host sums partials in fp32 and adds bo.

PSUM budget (8 banks): st pair [128,1024] x2bufs = 4, av [128,1024] = 2,
denominator [128,512] = 1, outproj/bcast/qkv rotate through 1.
"""

import numpy as np

import concourse.tile as tile_mod
from concourse import bacc, mybir
from concourse.bass_utils import run_bass_kernel_spmd
from concourse.vector_clock import ScopedClock, VectorClock

F32 = mybir.dt.float32
BF16 = mybir.dt.bfloat16
I16 = mybir.dt.int16

S, D, H, DK = 2048, 1024, 16, 64
P = 128
NCORES = 8

# Schraudolph exp on the DVE emitting bf16 via int16 bitcast:
# bf16(exp(x/8)) ~= bitcast<bf16>(int16(x * A_S + B_S)).
A_S = 0.125 * (2.0**23 / 0.6931471805599453) / 65536.0
B_S = (127 * 2.0**23 - 366000.0) / 65536.0 - 3.3

# Steps whose exp runs on the DVE (Schraudolph) instead of ACT, balanced
# against each engine's other work per phase: phase 0 ACT also evacuates
# q/k/v projections (DVE 18/32); phase 1 ACT takes unnorm + half the
# outproj evacuations (DVE 14/32).
DVE_STEPS = (
    {s for s in range(32) if (s * 18) // 32 != ((s + 1) * 18) // 32}
    | {32 + s for s in range(32) if (s * 14) // 32 != ((s + 1) * 14) // 32}
)


def _patched_drain_and_barrier(self, tick_clock, wait_clock):
    """This container's walrus build caps CTRL-type instructions at one sem
    wait; emit one Drain per outstanding proc instead."""
    gc = tick_clock.global_clock
    vec = list(gc)
    for i, t in enumerate(vec):
        if t <= 0:
            continue
        pv = [0] * len(vec)
        pv[i] = t
        d = self.nc.sync.drain()
        wait_clock.add_sem_waits(d.ins, ScopedClock({None: VectorClock(pv)}))

    self.nc.all_engine_barrier()
    assert self.sems is not None
    popped = self.nc._tile_sem_poison_stack.pop()
    assert popped is self._sem_poison
    self.nc.clear_and_free_semaphores(list(self.sems.allocated().values()))
    self.nc.all_engine_barrier()


tile_mod.TileContext._drain_and_barrier = _patched_drain_and_barrier


def _build_nc():
    from contextlib import ExitStack

    tile = tile_mod
    nc = bacc.Bacc(None)

    # et/wqkv come pre-arranged pi-major and chunk-major from the host so
    # every DMA line is >=3KB contiguous per partition (near-peak HBM bw).
    et = nc.declare_dram_parameter("et", [P, 4 * 8 * 512], BF16, isOutput=False)
    wqkv = nc.declare_dram_parameter("wqkv", [P, 8 * 384], BF16, isOutput=False)
    bq = nc.declare_dram_parameter("bq", [P, 1], F32, isOutput=False)
    bv = nc.declare_dram_parameter("bv", [P, 1], F32, isOutput=False)
    wo = nc.declare_dram_parameter("wo", [P, D], BF16, isOutput=False)
    out = nc.declare_dram_parameter("out", [D, S], BF16, isOutput=True)

    et4 = et.rearrange("p (c po s) -> p c po s", c=4, po=8)   # [128,4,8,512]
    wqkv3 = wqkv.rearrange("p (po c) -> p po c", po=8)        # [128, 8, 384]

    with tile.TileContext(nc) as tc, ExitStack() as ctx:
        consts = ctx.enter_context(tc.tile_pool(name="consts", bufs=1))
        qkv = ctx.enter_context(tc.tile_pool(name="qkv", bufs=1))
        utp = ctx.enter_context(tc.tile_pool(name="ut", bufs=6))
        headsp = ctx.enter_context(tc.tile_pool(name="heads", bufs=2))
        normp = ctx.enter_context(tc.tile_pool(name="norm", bufs=4))
        outp = ctx.enter_context(tc.tile_pool(name="outp", bufs=4))
        psum = ctx.enter_context(tc.tile_pool(name="psum", bufs=1, space="PSUM"))

        # ---- input DMAs: et chunks alternate sync/gpsimd; weights on
        # scalar. The DMA queues have ~5us startup and ~100-150 GB/s
        # effective rate, so chunk 0 and wqkv are split into per-dc pieces
        # that the first k0/q0 matmuls can chase. ------------------------
        et_sb = consts.tile([P, 8, S], BF16)
        rings = [nc.sync, nc.gpsimd]
        wqkv_sb = consts.tile([P, 8, 6 * DK], BF16)
        for pc in range(4):
            rings[pc % 2].dma_start(
                et_sb[:, 2 * pc : 2 * pc + 2, 0:512], et4[:, 0, 2 * pc : 2 * pc + 2]
            )
            nc.scalar.dma_start(
                wqkv_sb[:, 2 * pc : 2 * pc + 2, :], wqkv3[:, 2 * pc : 2 * pc + 2]
            )
        for c in range(1, 4):
            s0 = c * 512
            rings[c % 2].dma_start(et_sb[:, :, s0 : s0 + 512], et4[:, c])
        bq_sb = consts.tile([P, 1], F32)
        nc.scalar.dma_start(bq_sb[:], bq[:])
        bv_sb = consts.tile([P, 1], F32)
        nc.scalar.dma_start(bv_sb[:], bv[:])
        wo_sb = consts.tile([P, D], BF16)
        nc.scalar.dma_start(wo_sb[:], wo[:])
        warm_sb = consts.tile([P, 512], BF16)
        nc.vector.memset(warm_sb[:], 0.25)
        ones_sb = consts.tile([P, 1], BF16)
        nc.vector.memset(ones_sb[:], 1.0)
        onesf_sb = consts.tile([P, 64], F32)
        nc.vector.memset(onesf_sb[:], 1.0)

        # ACT exp-table preload (2.7us) during the DMA window.
        dummy_sb = qkv.tile([1, 8], BF16)
        nc.scalar.activation(
            dummy_sb[:], warm_sb[0:1, 0:8], mybir.ActivationFunctionType.Exp
        )

        # HAM warmup while input DMAs are in flight (don't overshoot — these
        # sit ahead of k0/q0 in the PE queue).
        def warm_mm():
            pw = psum.tile([P, 1024], F32, tag="st", bufs=2, name="warm")
            nc.tensor.matmul(
                pw[:, 0:512], warm_sb[:, 0:128], warm_sb[:], start=True, stop=True
            )

        for r in range(10):
            warm_mm()

        qt_sb = qkv.tile([P, S], BF16)
        kt_sb = qkv.tile([P, S], BF16)
        vt_sb = qkv.tile([P, S], BF16)
        v_sb = qkv.tile([P, 16, P], BF16)

        # ---- deferred-emission machinery: two queues. PE-side work is
        # drained BEFORE each step's st pair; engine-side work (psum
        # evacuations, norms) AFTER the step's exp, so evacuations never
        # sit ahead of an exp in the ACT/DVE FIFOs (convoy avoidance). ----
        pending_pe = []
        pending_eng = []
        seq_n = 0
        step = 0

        def push_pe(delay, fn):
            nonlocal seq_n
            pending_pe.append((step + delay, seq_n, fn))
            seq_n += 1

        def push_eng(delay, fn):
            nonlocal seq_n
            pending_eng.append((step + delay, seq_n, fn))
            seq_n += 1

        def drain(q):
            q.sort()
            while q and q[0][0] <= step:
                _, _, fn = q.pop(0)
                fn()

        # ---- QKV projection units (interleaved into attention steps) ----
        def qk_evac(which, c, ps):
            def fn():
                s0 = c * 512
                dst = qt_sb if which == 0 else kt_sb
                if which == 0:
                    nc.scalar.activation(
                        dst[:, s0 : s0 + 512], ps[:],
                        mybir.ActivationFunctionType.Identity,
                        bias=bq_sb[:, 0:1],
                    )
                else:
                    nc.scalar.activation(
                        dst[:, s0 : s0 + 512], ps[:],
                        mybir.ActivationFunctionType.Identity,
                    )
            return fn

        def emit_qk(which, c, inline_evac=True, interleave_warm=False):
            # which: 0=q, 1=k ; c: 512-col chunk of s
            s0 = c * 512
            ps = psum.tile([P, 512], F32, tag="op", bufs=1, name=f"qk{which}{c}")
            for dc in range(8):
                nc.tensor.matmul(
                    ps[:],
                    wqkv_sb[:, dc, which * 128 : which * 128 + 128],
                    et_sb[:, dc, s0 : s0 + 512],
                    start=(dc == 0),
                    stop=(dc == 7),
                )
                if interleave_warm and dc % 2 == 1:
                    # keep HAM fed while the dc chains chase the input DMA
                    warm_mm()
            if inline_evac:
                qk_evac(which, c, ps)()
            else:
                push_eng(0, qk_evac(which, c, ps))

        def emit_vt(c):
            # V^T [128 v, 512 t] for chunk c: one 8-dc accumulation chain,
            # ACT evac with the per-v bias, then a DMA transpose into the
            # [t, tb, v] layout the AV stationaries need.
            s0 = c * 512
            psv = psum.tile([P, 512], F32, tag="op", bufs=1, name=f"v{c}")
            for dc in range(8):
                nc.tensor.matmul(
                    psv[:],
                    wqkv_sb[:, dc, 256:384],
                    et_sb[:, dc, s0 : s0 + 512],
                    start=(dc == 0),
                    stop=(dc == 7),
                )

            def evac():
                nc.scalar.activation(
                    vt_sb[:, s0 : s0 + 512], psv[:],
                    mybir.ActivationFunctionType.Identity,
                    bias=bv_sb[:, 0:1],
                )
                nc.sync.dma_start_transpose(
                    v_sb[:, 4 * c : 4 * c + 4, :], vt_sb[:, s0 : s0 + 512]
                )
            push_eng(0, evac)

        QKV_UNITS = {
            0: [("q", 1)],
            1: [("vt", 0)],
            4: [("k", 1)],
            6: [("vt", 1)],
            8: [("k", 2)],
            10: [("vt", 2)],
            12: [("k", 3)],
            14: [("vt", 3)],
            18: [("q", 2)],
            22: [("q", 3)],
        }

        def emit_unit(u):
            kind, c = u
            if kind == "q":
                emit_qk(0, c, inline_evac=False)
            elif kind == "k":
                emit_qk(1, c, inline_evac=False)
            else:
                emit_vt(c)

        def mk_av(ut, tb, n0, av):
            def fn():
                nc0 = n0 * 512
                nc.tensor.matmul(
                    av[0:64, nc0 : nc0 + 512],
                    v_sb[:, tb, 0:64],
                    ut[:, 0:512],
                    start=(tb == 0), stop=(tb == 15),
                    skip_group_check=True,
                )
                nc.tensor.matmul(
                    av[64:128, nc0 : nc0 + 512],
                    v_sb[:, tb, 64:128],
                    ut[:, 512:1024],
                    start=(tb == 0), stop=(tb == 15),
                    skip_group_check=True,
                )
            return fn

        def mk_quad(tb, ut0, ut1, dn):
            def fn():
                for rowp, ut, uc in (
                    (0, ut0, 0), (32, ut0, 512), (64, ut1, 0), (96, ut1, 512)
                ):
                    nc.tensor.matmul(
                        dn[rowp : rowp + 1, :],
                        ones_sb[:, 0:1],
                        ut[:, uc : uc + 512],
                        start=(tb == 0), stop=(tb == 15),
                        skip_group_check=True,
                        tile_position=(0, rowp),
                    )
            return fn

        def emit_op_chunk(p, heads_sb, blk, ch, evac, ring, tag="op"):
            # PE-side: the projection matmul. Engine-side (next step, after
            # that step's exp): the psum evacuation + output DMA.
            c0 = blk * P
            s0 = ch * 512
            ps = psum.tile([P, 512], F32, tag=tag, bufs=1, name=f"op{p}{blk}{ch}")
            nc.tensor.matmul(
                ps[:], wo_sb[:, c0 : c0 + P], heads_sb[:, s0 : s0 + 512],
                start=True, stop=True,
            )

            def ev():
                ot = outp.tile([P, 512], BF16, tag="out")
                if evac == "dve":
                    nc.vector.tensor_copy(ot[:], ps[:])
                else:
                    nc.scalar.activation(
                        ot[:], ps[:], mybir.ActivationFunctionType.Identity
                    )
                ring.dma_start(
                    out[c0 : c0 + P, p * 1024 + s0 : p * 1024 + s0 + 512], ot[:]
                )
            push_eng(1, ev)

        def queue_phase_end(p, av, dn, heads_sb, last):
            rsb = normp.tile([97, 512], F32, tag="rsb", name=f"rsb{p}")
            unnorm = headsp.tile([P, 1024], F32, tag="unnorm", name=f"un{p}")
            rps = {}

            def chain_a():
                # ACT: unnormalized values out of the av bank.
                nc.scalar.activation(
                    unnorm[:], av[:], mybir.ActivationFunctionType.Identity
                )
                # DVE: denominators out of the quad bank + reciprocal.
                dsb = normp.tile([97, 512], F32, tag="dsb", name=f"dsb{p}")
                nc.vector.tensor_copy(dsb[:], dn[0:97, :])
                nc.vector.reciprocal_approx_fast(rsb[:], dsb[:])

            def mk_bcast(n0):
                def fn():
                    # rows: n0=0 -> (0 h0, 32 h1); n0=1 -> (64 h0, 96 h1)
                    t = psum.tile(
                        [P, 512], F32, tag="op", bufs=1, name=f"rps{p}{n0}"
                    )
                    rps[n0] = t
                    for hh, rowp in ((0, 64 * n0), (1, 64 * n0 + 32)):
                        nc.tensor.matmul(
                            t[hh * 64 : hh * 64 + 64, :],
                            onesf_sb[rowp : rowp + 1, :],
                            rsb[rowp : rowp + 1, :],
                            start=True, stop=True,
                            skip_group_check=True,
                            tile_position=(rowp, hh * 64),
                        )
                return fn

            def mk_norm(n0):
                def fn():
                    nc0 = n0 * 512
                    nc.vector.tensor_tensor(
                        heads_sb[:, nc0 : nc0 + 512],
                        unnorm[:, nc0 : nc0 + 512],
                        rps[n0][:],
                        mybir.AluOpType.mult,
                    )
                return fn

            if last:
                # av(63)/quad(15) are pending with due=65; the normalize
                # chain must be emitted after them.
                push_eng(2, chain_a)
                push_pe(3, mk_bcast(0))
                push_eng(3, mk_norm(0))
                push_pe(4, mk_bcast(1))
                push_eng(4, mk_norm(1))
                # tail outproj: pairs (blk, both ch) rotating st+av banks,
                # evac alternating ACT/DVE, DMA over four rings. Junk
                # matmuls into the op bank keep the HAM clock warm through
                # the evac-paced stretch.
                def tail_junk():
                    pj = psum.tile([P, 512], F32, tag="op", bufs=1, name="tj")
                    nc.tensor.matmul(
                        pj[:], warm_sb[:, 0:128], warm_sb[:],
                        start=True, stop=True,
                    )
                tail_rings = [nc.sync, nc.gpsimd, nc.scalar, nc.gpsimd]
                # bridge the reciprocal/broadcast latency so HAM never sees
                # a >3.4us PE idle at the tail start
                push_pe(2, tail_junk)
                push_pe(3, tail_junk)
                push_pe(4, tail_junk)
                for blk in range(8):
                    tag = "av" if blk % 3 == 2 else "st"
                    tl = psum.tile(
                        [P, 1024], F32, tag=tag, bufs=(1 if tag == "av" else 2),
                        name=f"tp{blk}"
                    )

                    def tail_mms(blk=blk, tl=tl):
                        c0 = blk * P
                        for ch in range(2):
                            nc.tensor.matmul(
                                tl[:, ch * 512 : ch * 512 + 512],
                                wo_sb[:, c0 : c0 + P],
                                heads_sb[:, ch * 512 : ch * 512 + 512],
                                start=True, stop=True,
                                skip_group_check=True,
                            )

                    def tail_evac(blk=blk, tl=tl):
                        # halves on different engines so both engines work
                        # every pair and each chunk's DMA starts sooner
                        c0 = blk * P
                        ot = outp.tile([P, 1024], BF16, tag="out2", bufs=4)
                        for ch in range(2):
                            h = tl[:, ch * 512 : ch * 512 + 512]
                            o = ot[:, ch * 512 : ch * 512 + 512]
                            if (blk + ch) % 2:
                                nc.vector.tensor_copy(o, h)
                            else:
                                nc.scalar.activation(
                                    o, h, mybir.ActivationFunctionType.Identity
                                )
                            tail_rings[(2 * blk + ch) % 4].dma_start(
                                out[c0 : c0 + P,
                                    1024 + ch * 512 : 1536 + ch * 512],
                                o,
                            )
                    push_pe(5 + blk // 2, tail_mms)
                    push_pe(5 + blk // 2, tail_junk)
                    push_eng(5 + blk // 2, tail_evac)
            else:
                push_eng(2, chain_a)
                push_pe(4, mk_bcast(0))
                push_eng(4, mk_norm(0))
                push_pe(5, mk_bcast(1))
                push_eng(5, mk_norm(1))
                for i, (blk, ch) in enumerate(
                    [(b, c) for b in range(8) for c in range(2)]
                ):
                    push_pe((lambda d: d + d % 2)(6 + (i * 7) // 4), (
                        lambda b=blk, c=ch, i=i: emit_op_chunk(
                            p, heads_sb, b, c,
                            evac=("dve" if i % 2 else "act"),
                            ring=rings[i % 2],
                        )
                    ))

        # ---- head: k0, q0 before the step loop -------------------------
        emit_qk(1, 0, interleave_warm=True)
        emit_qk(0, 0, interleave_warm=True)

        heads_tiles = {}
        for p in range(2):
            heads_tiles[p] = headsp.tile(
                [P, 1024], BF16, tag="heads", name=f"heads{p}"
            )

        # ---- main attention loop ---------------------------------------
        for p in range(2):
            qbase = p * 1024
            av = psum.tile([P, 1024], F32, tag="av", bufs=1, name=f"av{p}")
            dn = psum.tile([P, 512], F32, tag="dn", bufs=1, name=f"dn{p}")
            ut_prev = None
            for tb in range(16):
                for n0 in (0, 1):
                    drain(pending_pe)
                    for u in QKV_UNITS.get(step, []):
                        emit_unit(u)
                    st = psum.tile(
                        [P, 1024], F32, tag="st", bufs=2, name=f"st{p}{tb}{n0}"
                    )
                    t0 = tb * P
                    qc = qbase + n0 * 512
                    nc.tensor.matmul(
                        st[:, 0:512],
                        kt_sb[0:64, t0 : t0 + P],
                        qt_sb[0:64, qc : qc + 512],
                        start=True, stop=True,
                        skip_group_check=True,
                    )
                    nc.tensor.matmul(
                        st[:, 512:1024],
                        kt_sb[64:128, t0 : t0 + P],
                        qt_sb[64:128, qc : qc + 512],
                        start=True, stop=True,
                        skip_group_check=True,
                    )
                    if step in DVE_STEPS:
                        ei = utp.tile([P, 1024], I16, tag="ut", bufs=6)
                        nc.vector.tensor_scalar(
                            ei[:], st[:], A_S, B_S,
                            mybir.AluOpType.mult, mybir.AluOpType.add,
                        )
                        ut = ei.bitcast(BF16)
                    else:
                        utt = utp.tile([P, 1024], BF16, tag="ut", bufs=6)
                        nc.scalar.activation(
                            utt[:], st[:],
                            mybir.ActivationFunctionType.Exp, scale=0.125,
                        )
                        ut = utt
                    # delays chosen so both av pairs and the quad of a tb
                    # land in ONE drain: [avA, avB, quad] emit adjacently,
                    # and their av->av / av->quad weight loads hide under
                    # the previous tiles' disjoint col-groups.
                    push_pe(4 - n0, mk_av(ut, tb, n0, av))
                    if n0 == 1:
                        push_pe(3, mk_quad(tb, ut_prev, ut, dn))
                    ut_prev = ut
                    drain(pending_eng)
                    step += 1
            queue_phase_end(p, av, dn, heads_tiles[p], last=(p == 1))

        # ---- tail drain ------------------------------------------------
        for _ in range(24):
            step += 1
            drain(pending_pe)
            drain(pending_eng)

    nc.finalize()
    return nc


_NC_CACHE = None


def _get_nc():
    global _NC_CACHE
    if _NC_CACHE is None:
        _NC_CACHE = _build_nc()
    return _NC_CACHE


def _make_in_maps(embeddings, Wq, bq, Wk, bk, Wv, bv, Wo, bo):
    import ml_dtypes

    bf16 = np.dtype(ml_dtypes.bfloat16)
    etT = embeddings.T.astype(bf16)  # [1024, 2048]
    # -> [128 pi, 4 chunk, 8 po, 512 s] so each chunk DMA reads 4KB
    # contiguous per partition.
    et = np.ascontiguousarray(
        etT.reshape(8, P, 4, 512).transpose(1, 2, 0, 3).reshape(P, 4 * 8 * 512)
    )
    in_maps = []
    for c in range(NCORES):
        hs = [2 * c, 2 * c + 1]
        wqkv = np.concatenate(
            [Wq[hs[0]], Wq[hs[1]], Wk[hs[0]], Wk[hs[1]], Wv[hs[0]], Wv[hs[1]]],
            axis=1,
        ).astype(bf16)  # [1024, 384]
        # -> [128 pi, 8 po, 384] contiguous per partition.
        wqkv = np.ascontiguousarray(
            wqkv.reshape(8, P, 384).transpose(1, 0, 2).reshape(P, 8 * 384)
        )
        bq_c = np.concatenate([bq[hs[0]], bq[hs[1]]]).astype(np.float32)[:, None]
        bv_c = np.concatenate([bv[hs[0]], bv[hs[1]]]).astype(np.float32)[:, None]
        bv_c = np.ascontiguousarray(bv_c)
        in_maps.append(
            {
                "et": et,
                "wqkv": wqkv,
                "bq": np.ascontiguousarray(bq_c),
                "bv": bv_c,
                "wo": np.ascontiguousarray(Wo[c * P : (c + 1) * P].astype(bf16)),
            }
        )
    return in_maps


def kernel(embeddings, Wq, bq, Wk, bk, Wv, bv, Wo, bo, **run_kwargs):
    """Full-input / full-output MHA. Shards across 8 NeuronCores internally."""
    nc = _get_nc()
    in_maps = _make_in_maps(
        np.asarray(embeddings, np.float32),
        np.asarray(Wq, np.float32),
        np.asarray(bq, np.float32),
        np.asarray(Wk, np.float32),
        np.asarray(bk, np.float32),
        np.asarray(Wv, np.float32),
        np.asarray(bv, np.float32),
        np.asarray(Wo, np.float32),
        np.asarray(bo, np.float32),
    )
    res = run_bass_kernel_spmd(nc, in_maps, list(range(NCORES)), **run_kwargs)
    acc = res.results[0]["out"].astype(np.float32)
    for r_ in res.results[1:]:
        acc += r_["out"].astype(np.float32)
    acc = acc.T + np.asarray(bo, np.float32)[None, :]
    return np.ascontiguousarray(acc)


if __name__ == "__main__":
    rng = np.random.default_rng(0)
    emb = rng.standard_normal((S, D), dtype=np.float32)
    mk = lambda *sh: (rng.standard_normal(sh, dtype=np.float32) * 0.02)
    o = kernel(
        embeddings=emb,
        Wq=mk(H, D, DK), bq=mk(H, DK),
        Wk=mk(H, D, DK), bk=mk(H, DK),
        Wv=mk(H, D, DK), bv=mk(H, DK),
        Wo=mk(H * DK, D), bo=mk(D),
    )
    print(o.shape, o.dtype)


# revision 7
# speedup vs baseline: 1.0438x; 1.0438x over previous
"""Trainium2 Bass kernel v4: 16-head MHA (S=2048, D=1024, Dk=Dv=64) on 8 cores.

Sharding: tensor-parallel over heads (2 heads/core) + row-sharded Wo
(fp-summed partials on host). Differences vs v3 baseline:

  - Both heads processed per phase (phase = query half). Score matmuls are
    K=64 ROW-TILED PAIRS (tile_position (0,0)/(64,0)) so the two heads'
    QK^T share the PE array concurrently -> 2x score throughput.
  - AV matmuls are M=64 COL-TILED PAIRS ((0,0)/(0,64)); the softmax
    denominator moves out of the AV stationary (no more M=65) into a
    4-tile M=1 "denominator quad" slot per t-block ((0,0)/(0,32)/(0,64)/
    (0,96)) -> AV+denominator costs 3 slots per t-block instead of 4.
  - QKV projection chunks are interleaved INTO the attention steps so the
    exp stream (the scarce resource: ACT ~114 G/s + DVE ~95 G/s) starts
    ~10us earlier and the PE never idles long enough for HAM to
    re-throttle the clock.
  - exp split ACT (table exp) / DVE (Schraudolph int16 bitcast) per-step.
  - Denominator reciprocal via one reciprocal_approx_fast over the quad
    psum rows (no DMA round-trip), broadcast by K=1 fp32 matmuls.
  - Output projection partials evacuated bf16 (ACT/DVE alternating),
    DMA'd per chunk; host sums partials in fp32 and adds bo.

PSUM budget (8 banks): st pair [128,1024] x2bufs = 4, av [128,1024] = 2,
denominator [128,512] = 1, outproj/bcast/qkv rotate through 1.
"""

import numpy as np

import concourse.tile as tile_mod
from concourse import bacc, mybir
from concourse.bass_utils import run_bass_kernel_spmd
from concourse.vector_clock import ScopedClock, VectorClock

F32 = mybir.dt.float32
BF16 = mybir.dt.bfloat16
I16 = mybir.dt.int16

S, D, H, DK = 2048, 1024, 16, 64
P = 128
NCORES = 8

# Schraudolph exp on the DVE emitting bf16 via int16 bitcast:
# bf16(exp(x/8)) ~= bitcast<bf16>(int16(x * A_S + B_S)).
A_S = 0.125 * (2.0**23 / 0.6931471805599453) / 65536.0
B_S = (127 * 2.0**23 - 366000.0) / 65536.0 - 3.3

# Steps whose exp runs on the DVE (Schraudolph) instead of ACT, balanced
# against each engine's other work per phase: phase 0 ACT also evacuates
# q/k/v projections (DVE 18/32); phase 1 ACT takes unnorm + half the
# outproj evacuations (DVE 14/32).
DVE_STEPS = (
    {s for s in range(32) if (s * 18) // 32 != ((s + 1) * 18) // 32}
    | {32 + s for s in range(32) if (s * 14) // 32 != ((s + 1) * 14) // 32}
)


def _patched_drain_and_barrier(self, tick_clock, wait_clock):
    """This container's walrus build caps CTRL-type instructions at one sem
    wait; emit one Drain per outstanding proc instead."""
    gc = tick_clock.global_clock
    vec = list(gc)
    for i, t in enumerate(vec):
        if t <= 0:
            continue
        pv = [0] * len(vec)
        pv[i] = t
        d = self.nc.sync.drain()
        wait_clock.add_sem_waits(d.ins, ScopedClock({None: VectorClock(pv)}))

    self.nc.all_engine_barrier()
    assert self.sems is not None
    popped = self.nc._tile_sem_poison_stack.pop()
    assert popped is self._sem_poison
    self.nc.clear_and_free_semaphores(list(self.sems.allocated().values()))
    self.nc.all_engine_barrier()


tile_mod.TileContext._drain_and_barrier = _patched_drain_and_barrier


def _build_nc():
    from contextlib import ExitStack

    tile = tile_mod
    nc = bacc.Bacc(None)

    # et/wqkv come pre-arranged pi-major and chunk-major from the host so
    # every DMA line is >=3KB contiguous per partition (near-peak HBM bw).
    et = nc.declare_dram_parameter("et", [P, 4 * 8 * 512], BF16, isOutput=False)
    wqkv = nc.declare_dram_parameter("wqkv", [P, 8 * 384], BF16, isOutput=False)
    bq = nc.declare_dram_parameter("bq", [P, 1], F32, isOutput=False)
    bv = nc.declare_dram_parameter("bv", [P, 1], F32, isOutput=False)
    wo = nc.declare_dram_parameter("wo", [P, D], BF16, isOutput=False)
    out = nc.declare_dram_parameter("out", [D, S], BF16, isOutput=True)

    et4 = et.rearrange("p (c po s) -> p c po s", c=4, po=8)   # [128,4,8,512]
    wqkv3 = wqkv.rearrange("p (po c) -> p po c", po=8)        # [128, 8, 384]

    with tile.TileContext(nc) as tc, ExitStack() as ctx:
        consts = ctx.enter_context(tc.tile_pool(name="consts", bufs=1))
        qkv = ctx.enter_context(tc.tile_pool(name="qkv", bufs=1))
        utp = ctx.enter_context(tc.tile_pool(name="ut", bufs=6))
        headsp = ctx.enter_context(tc.tile_pool(name="heads", bufs=2))
        normp = ctx.enter_context(tc.tile_pool(name="norm", bufs=4))
        outp = ctx.enter_context(tc.tile_pool(name="outp", bufs=4))
        psum = ctx.enter_context(tc.tile_pool(name="psum", bufs=1, space="PSUM"))

        # ---- input DMAs: et chunks alternate sync/gpsimd; weights on
        # scalar. The DMA queues have ~5us startup and ~100-150 GB/s
        # effective rate, so chunk 0 and wqkv are split into per-dc pieces
        # that the first k0/q0 matmuls can chase. ------------------------
        et_sb = consts.tile([P, 8, S], BF16)
        rings = [nc.sync, nc.gpsimd]
        wqkv_sb = consts.tile([P, 8, 6 * DK], BF16)
        for pc in range(4):
            rings[pc % 2].dma_start(
                et_sb[:, 2 * pc : 2 * pc + 2, 0:512], et4[:, 0, 2 * pc : 2 * pc + 2]
            )
            nc.scalar.dma_start(
                wqkv_sb[:, 2 * pc : 2 * pc + 2, :], wqkv3[:, 2 * pc : 2 * pc + 2]
            )
        for c in range(1, 4):
            s0 = c * 512
            rings[c % 2].dma_start(et_sb[:, :, s0 : s0 + 512], et4[:, c])
        bq_sb = consts.tile([P, 1], F32)
        nc.scalar.dma_start(bq_sb[:], bq[:])
        bv_sb = consts.tile([P, 1], F32)
        nc.scalar.dma_start(bv_sb[:], bv[:])
        wo_sb = consts.tile([P, D], BF16)
        nc.scalar.dma_start(wo_sb[:], wo[:])
        warm_sb = consts.tile([P, 512], BF16)
        nc.vector.memset(warm_sb[:], 0.25)
        ones_sb = consts.tile([P, 1], BF16)
        nc.vector.memset(ones_sb[:], 1.0)
        onesf_sb = consts.tile([P, 64], F32)
        nc.vector.memset(onesf_sb[:], 1.0)

        # ACT exp-table preload (2.7us) during the DMA window.
        dummy_sb = qkv.tile([1, 8], BF16)
        nc.scalar.activation(
            dummy_sb[:], warm_sb[0:1, 0:8], mybir.ActivationFunctionType.Exp
        )

        # HAM warmup while input DMAs are in flight (don't overshoot — these
        # sit ahead of k0/q0 in the PE queue).
        def warm_mm():
            pw = psum.tile([P, 1024], F32, tag="st", bufs=2, name="warm")
            nc.tensor.matmul(
                pw[:, 0:512], warm_sb[:, 0:128], warm_sb[:], start=True, stop=True
            )

        for r in range(10):
            warm_mm()

        qt_sb = qkv.tile([P, S], BF16)
        kt_sb = qkv.tile([P, S], BF16)
        vt_sb = qkv.tile([P, S], BF16)
        v_sb = qkv.tile([P, 16, P], BF16)

        # ---- deferred-emission machinery: two queues. PE-side work is
        # drained BEFORE each step's st pair; engine-side work (psum
        # evacuations, norms) AFTER the step's exp, so evacuations never
        # sit ahead of an exp in the ACT/DVE FIFOs (convoy avoidance). ----
        pending_pe = []
        pending_eng = []
        seq_n = 0
        step = 0

        def push_pe(delay, fn):
            nonlocal seq_n
            pending_pe.append((step + delay, seq_n, fn))
            seq_n += 1

        def push_eng(delay, fn):
            nonlocal seq_n
            pending_eng.append((step + delay, seq_n, fn))
            seq_n += 1

        def drain(q):
            q.sort()
            while q and q[0][0] <= step:
                _, _, fn = q.pop(0)
                fn()

        # ---- QKV projection units (interleaved into attention steps) ----
        def qk_evac(which, c, ps):
            def fn():
                s0 = c * 512
                dst = qt_sb if which == 0 else kt_sb
                if which == 0:
                    nc.scalar.activation(
                        dst[:, s0 : s0 + 512], ps[:],
                        mybir.ActivationFunctionType.Identity,
                        bias=bq_sb[:, 0:1],
                    )
                else:
                    nc.scalar.activation(
                        dst[:, s0 : s0 + 512], ps[:],
                        mybir.ActivationFunctionType.Identity,
                    )
            return fn

        def emit_qk(which, c, inline_evac=True, interleave_warm=False):
            # which: 0=q, 1=k ; c: 512-col chunk of s
            s0 = c * 512
            ps = psum.tile([P, 512], F32, tag="op", bufs=1, name=f"qk{which}{c}")
            for dc in range(8):
                nc.tensor.matmul(
                    ps[:],
                    wqkv_sb[:, dc, which * 128 : which * 128 + 128],
                    et_sb[:, dc, s0 : s0 + 512],
                    start=(dc == 0),
                    stop=(dc == 7),
                )
                if interleave_warm and (dc % 2 == 1 or which == 1):
                    # keep HAM fed while the dc chains chase the input DMA
                    warm_mm()
            if inline_evac:
                qk_evac(which, c, ps)()
            else:
                push_eng(0, qk_evac(which, c, ps))

        def emit_vt(c):
            # V^T [128 v, 512 t] for chunk c: one 8-dc accumulation chain,
            # ACT evac with the per-v bias, then a DMA transpose into the
            # [t, tb, v] layout the AV stationaries need.
            s0 = c * 512
            psv = psum.tile([P, 512], F32, tag="op", bufs=1, name=f"v{c}")
            for dc in range(8):
                nc.tensor.matmul(
                    psv[:],
                    wqkv_sb[:, dc, 256:384],
                    et_sb[:, dc, s0 : s0 + 512],
                    start=(dc == 0),
                    stop=(dc == 7),
                )

            def evac():
                nc.scalar.activation(
                    vt_sb[:, s0 : s0 + 512], psv[:],
                    mybir.ActivationFunctionType.Identity,
                    bias=bv_sb[:, 0:1],
                )
                nc.sync.dma_start_transpose(
                    v_sb[:, 4 * c : 4 * c + 4, :], vt_sb[:, s0 : s0 + 512]
                )
            push_eng(0, evac)

        QKV_UNITS = {
            0: [("q", 1)],
            1: [("vt", 0)],
            4: [("k", 1)],
            6: [("vt", 1)],
            8: [("k", 2)],
            10: [("vt", 2)],
            12: [("k", 3)],
            14: [("vt", 3)],
            18: [("q", 2)],
            22: [("q", 3)],
        }

        def emit_unit(u):
            kind, c = u
            if kind == "q":
                emit_qk(0, c, inline_evac=False)
            elif kind == "k":
                emit_qk(1, c, inline_evac=False)
            else:
                emit_vt(c)

        def mk_av(ut, tb, n0, av):
            def fn():
                nc0 = n0 * 512
                nc.tensor.matmul(
                    av[0:64, nc0 : nc0 + 512],
                    v_sb[:, tb, 0:64],
                    ut[:, 0:512],
                    start=(tb == 0), stop=(tb == 15),
                    skip_group_check=True,
                )
                nc.tensor.matmul(
                    av[64:128, nc0 : nc0 + 512],
                    v_sb[:, tb, 64:128],
                    ut[:, 512:1024],
                    start=(tb == 0), stop=(tb == 15),
                    skip_group_check=True,
                )
            return fn

        def mk_quad(tb, ut0, ut1, dn):
            def fn():
                for rowp, ut, uc in (
                    (0, ut0, 0), (32, ut0, 512), (64, ut1, 0), (96, ut1, 512)
                ):
                    nc.tensor.matmul(
                        dn[rowp : rowp + 1, :],
                        ones_sb[:, 0:1],
                        ut[:, uc : uc + 512],
                        start=(tb == 0), stop=(tb == 15),
                        skip_group_check=True,
                        tile_position=(0, rowp),
                    )
            return fn

        def emit_op_chunk(p, heads_sb, blk, ch, evac, ring, tag="op"):
            # PE-side: the projection matmul. Engine-side (next step, after
            # that step's exp): the psum evacuation + output DMA.
            c0 = blk * P
            s0 = ch * 512
            ps = psum.tile([P, 512], F32, tag=tag, bufs=1, name=f"op{p}{blk}{ch}")
            nc.tensor.matmul(
                ps[:], wo_sb[:, c0 : c0 + P], heads_sb[:, s0 : s0 + 512],
                start=True, stop=True,
            )

            def ev():
                ot = outp.tile([P, 512], BF16, tag="out")
                if evac == "dve":
                    nc.vector.tensor_copy(ot[:], ps[:])
                else:
                    nc.scalar.activation(
                        ot[:], ps[:], mybir.ActivationFunctionType.Identity
                    )
                ring.dma_start(
                    out[c0 : c0 + P, p * 1024 + s0 : p * 1024 + s0 + 512], ot[:]
                )
            push_eng(1, ev)

        def queue_phase_end(p, av, dn, heads_sb, last):
            rsb = normp.tile([97, 512], F32, tag="rsb", name=f"rsb{p}")
            unnorm = headsp.tile([P, 1024], F32, tag="unnorm", name=f"un{p}")
            rps = {}

            def chain_a():
                # ACT: unnormalized values out of the av bank.
                nc.scalar.activation(
                    unnorm[:], av[:], mybir.ActivationFunctionType.Identity
                )
                # DVE: denominators out of the quad bank + reciprocal.
                dsb = normp.tile([97, 512], F32, tag="dsb", name=f"dsb{p}")
                nc.vector.tensor_copy(dsb[:], dn[0:97, :])
                nc.vector.reciprocal_approx_fast(rsb[:], dsb[:])

            def mk_bcast(n0):
                def fn():
                    # rows: n0=0 -> (0 h0, 32 h1); n0=1 -> (64 h0, 96 h1)
                    t = psum.tile(
                        [P, 512], F32, tag="op", bufs=1, name=f"rps{p}{n0}"
                    )
                    rps[n0] = t
                    for hh, rowp in ((0, 64 * n0), (1, 64 * n0 + 32)):
                        nc.tensor.matmul(
                            t[hh * 64 : hh * 64 + 64, :],
                            onesf_sb[rowp : rowp + 1, :],
                            rsb[rowp : rowp + 1, :],
                            start=True, stop=True,
                            skip_group_check=True,
                            tile_position=(rowp, hh * 64),
                        )
                return fn

            def mk_norm(n0):
                def fn():
                    nc0 = n0 * 512
                    nc.vector.tensor_tensor(
                        heads_sb[:, nc0 : nc0 + 512],
                        unnorm[:, nc0 : nc0 + 512],
                        rps[n0][:],
                        mybir.AluOpType.mult,
                    )
                return fn

            if last:
                # av(63)/quad(15) are pending with due=65; the normalize
                # chain must be emitted after them.
                push_eng(2, chain_a)
                push_pe(3, mk_bcast(0))
                push_eng(3, mk_norm(0))
                push_pe(4, mk_bcast(1))
                push_eng(4, mk_norm(1))
                # tail outproj: pairs (blk, both ch) rotating st+av banks,
                # evac alternating ACT/DVE, DMA over four rings. Junk
                # matmuls into the op bank keep the HAM clock warm through
                # the evac-paced stretch.
                def tail_junk():
                    pj = psum.tile([P, 512], F32, tag="op", bufs=1, name="tj")
                    nc.tensor.matmul(
                        pj[:], warm_sb[:, 0:128], warm_sb[:],
                        start=True, stop=True,
                    )
                tail_rings = [nc.sync, nc.gpsimd, nc.scalar, nc.gpsimd]
                # bridge the reciprocal/broadcast latency so HAM never sees
                # a >3.4us PE idle at the tail start
                push_pe(2, tail_junk)
                push_pe(3, tail_junk)
                push_pe(4, tail_junk)
                push_pe(5, tail_junk)
                push_pe(6, tail_junk)
                for blk in range(8):
                    tag = "av" if blk % 3 == 2 else "st"
                    tl = psum.tile(
                        [P, 1024], F32, tag=tag, bufs=(1 if tag == "av" else 2),
                        name=f"tp{blk}"
                    )

                    def tail_mms(blk=blk, tl=tl):
                        c0 = blk * P
                        for ch in range(2):
                            nc.tensor.matmul(
                                tl[:, ch * 512 : ch * 512 + 512],
                                wo_sb[:, c0 : c0 + P],
                                heads_sb[:, ch * 512 : ch * 512 + 512],
                                start=True, stop=True,
                                skip_group_check=True,
                            )

                    def tail_evac(blk=blk, tl=tl):
                        # halves on different engines so both engines work
                        # every pair and each chunk's DMA starts sooner
                        c0 = blk * P
                        ot = outp.tile([P, 1024], BF16, tag="out2", bufs=4)
                        for ch in range(2):
                            h = tl[:, ch * 512 : ch * 512 + 512]
                            o = ot[:, ch * 512 : ch * 512 + 512]
                            if (blk + ch) % 2:
                                nc.vector.tensor_copy(o, h)
                            else:
                                nc.scalar.activation(
                                    o, h, mybir.ActivationFunctionType.Identity
                                )
                            tail_rings[(2 * blk + ch) % 4].dma_start(
                                out[c0 : c0 + P,
                                    1024 + ch * 512 : 1536 + ch * 512],
                                o,
                            )
                    push_pe(5 + blk // 2, tail_mms)
                    push_pe(5 + blk // 2, tail_junk)
                    push_eng(5 + blk // 2, tail_evac)
            else:
                push_eng(2, chain_a)
                push_pe(4, mk_bcast(0))
                push_eng(4, mk_norm(0))
                push_pe(5, mk_bcast(1))
                push_eng(5, mk_norm(1))
                for i, (blk, ch) in enumerate(
                    [(b, c) for b in range(8) for c in range(2)]
                ):
                    push_pe((lambda d: d + d % 2)(6 + (i * 7) // 4), (
                        lambda b=blk, c=ch, i=i: emit_op_chunk(
                            p, heads_sb, b, c,
                            evac=("dve" if i % 2 else "act"),
                            ring=rings[i % 2],
                        )
                    ))

        # ---- head: k0, q0 before the step loop -------------------------
        emit_qk(1, 0, interleave_warm=True)
        emit_qk(0, 0, interleave_warm=True)

        heads_tiles = {}
        for p in range(2):
            heads_tiles[p] = headsp.tile(
                [P, 1024], BF16, tag="heads", name=f"heads{p}"
            )

        # ---- main attention loop ---------------------------------------
        for p in range(2):
            qbase = p * 1024
            av = psum.tile([P, 1024], F32, tag="av", bufs=1, name=f"av{p}")
            dn = psum.tile([P, 512], F32, tag="dn", bufs=1, name=f"dn{p}")
            ut_prev = None
            for tb in range(16):
                for n0 in (0, 1):
                    drain(pending_pe)
                    for u in QKV_UNITS.get(step, []):
                        emit_unit(u)
                    st = psum.tile(
                        [P, 1024], F32, tag="st", bufs=2, name=f"st{p}{tb}{n0}"
                    )
                    t0 = tb * P
                    qc = qbase + n0 * 512
                    nc.tensor.matmul(
                        st[:, 0:512],
                        kt_sb[0:64, t0 : t0 + P],
                        qt_sb[0:64, qc : qc + 512],
                        start=True, stop=True,
                        skip_group_check=True,
                    )
                    nc.tensor.matmul(
                        st[:, 512:1024],
                        kt_sb[64:128, t0 : t0 + P],
                        qt_sb[64:128, qc : qc + 512],
                        start=True, stop=True,
                        skip_group_check=True,
                    )
                    if step in DVE_STEPS:
                        ei = utp.tile([P, 1024], I16, tag="ut", bufs=6)
                        nc.vector.tensor_scalar(
                            ei[:], st[:], A_S, B_S,
                            mybir.AluOpType.mult, mybir.AluOpType.add,
                        )
                        ut = ei.bitcast(BF16)
                    else:
                        utt = utp.tile([P, 1024], BF16, tag="ut", bufs=6)
                        nc.scalar.activation(
                            utt[:], st[:],
                            mybir.ActivationFunctionType.Exp, scale=0.125,
                        )
                        ut = utt
                    # delays chosen so both av pairs and the quad of a tb
                    # land in ONE drain: [avA, avB, quad] emit adjacently,
                    # and their av->av / av->quad weight loads hide under
                    # the previous tiles' disjoint col-groups.
                    push_pe(4 - n0, mk_av(ut, tb, n0, av))
                    if n0 == 1:
                        push_pe(3, mk_quad(tb, ut_prev, ut, dn))
                    ut_prev = ut
                    drain(pending_eng)
                    step += 1
            queue_phase_end(p, av, dn, heads_tiles[p], last=(p == 1))

        # ---- tail drain ------------------------------------------------
        for _ in range(24):
            step += 1
            drain(pending_pe)
            drain(pending_eng)

    nc.finalize()
    return nc


_NC_CACHE = None


def _get_nc():
    global _NC_CACHE
    if _NC_CACHE is None:
        _NC_CACHE = _build_nc()
    return _NC_CACHE


def _make_in_maps(embeddings, Wq, bq, Wk, bk, Wv, bv, Wo, bo):
    import ml_dtypes

    bf16 = np.dtype(ml_dtypes.bfloat16)
    etT = embeddings.T.astype(bf16)  # [1024, 2048]
    # -> [128 pi, 4 chunk, 8 po, 512 s] so each chunk DMA reads 4KB
    # contiguous per partition.
    et = np.ascontiguousarray(
        etT.reshape(8, P, 4, 512).transpose(1, 2, 0, 3).reshape(P, 4 * 8 * 512)
    )
    in_maps = []
    for c in range(NCORES):
        hs = [2 * c, 2 * c + 1]
        wqkv = np.concatenate(
            [Wq[hs[0]], Wq[hs[1]], Wk[hs[0]], Wk[hs[1]], Wv[hs[0]], Wv[hs[1]]],
            axis=1,
        ).astype(bf16)  # [1024, 384]
        # -> [128 pi, 8 po, 384] contiguous per partition.
        wqkv = np.ascontiguousarray(
            wqkv.reshape(8, P, 384).transpose(1, 0, 2).reshape(P, 8 * 384)
        )
        bq_c = np.concatenate([bq[hs[0]], bq[hs[1]]]).astype(np.float32)[:, None]
        bv_c = np.concatenate([bv[hs[0]], bv[hs[1]]]).astype(np.float32)[:, None]
        bv_c = np.ascontiguousarray(bv_c)
        in_maps.append(
            {
                "et": et,
                "wqkv": wqkv,
                "bq": np.ascontiguousarray(bq_c),
                "bv": bv_c,
                "wo": np.ascontiguousarray(Wo[c * P : (c + 1) * P].astype(bf16)),
            }
        )
    return in_maps


def kernel(embeddings, Wq, bq, Wk, bk, Wv, bv, Wo, bo, **run_kwargs):
    """Full-input / full-output MHA. Shards across 8 NeuronCores internally."""
    nc = _get_nc()
    in_maps = _make_in_maps(
        np.asarray(embeddings, np.float32),
        np.asarray(Wq, np.float32),
        np.asarray(bq, np.float32),
        np.asarray(Wk, np.float32),
        np.asarray(bk, np.float32),
        np.asarray(Wv, np.float32),
        np.asarray(bv, np.float32),
        np.asarray(Wo, np.float32),
        np.asarray(bo, np.float32),
    )
    res = run_bass_kernel_spmd(nc, in_maps, list(range(NCORES)), **run_kwargs)
    acc = res.results[0]["out"].astype(np.float32)
    for r_ in res.results[1:]:
        acc += r_["out"].astype(np.float32)
    acc = acc.T + np.asarray(bo, np.float32)[None, :]
    return np.ascontiguousarray(acc)


if __name__ == "__main__":
    rng = np.random.default_rng(0)
    emb = rng.standard_normal((S, D), dtype=np.float32)
    mk = lambda *sh: (rng.standard_normal(sh, dtype=np.float32) * 0.02)
    o = kernel(
        embeddings=emb,
        Wq=mk(H, D, DK), bq=mk(H, DK),
        Wk=mk(H, D, DK), bk=mk(H, DK),
        Wv=mk(H, D, DK), bv=mk(H, DK),
        Wo=mk(H * DK, D), bo=mk(D),
    )
    print(o.shape, o.dtype)
